# revision 26
# baseline (speedup 1.0000x reference)
"""GAT layer kernel for Trainium2, sharded across 8 NeuronCores.

Math: reference computes
    h = x @ W.T;  e_ij = (h @ a1)[i] + (h @ a2)[j];  mask by adj;
    softmax over j; out = attn @ h.
Because e_i is constant along the softmax axis it cancels, so with
w_j = exp(h_j . a2):
    out[i] = sum_j adj[i,j] * w_j * h[j] / sum_j adj[i,j] * w_j
a1 is mathematically irrelevant.

Design (v10, fused single pass):
  * adjacency is packed host-side to fp8e4 BIT PATTERNS (0x00 / 0x38 =
    1.0): 1 byte/entry (8.4 MB/core vs 33.5 int32), streamed pure-copy
    on SWDGE, interleaved just-in-time with the bf16 x stream.
  * ONE fused loop over 32 j-super-chunks: each iteration computes the
    next quarter of h (2 j-chunks, bf16 matmuls into a 2-bank PSUM
    scratch), exps the e column read straight out of PSUM, converts
    w*h to the fp8 stationaries in a single DVE op (w broadcast via a
    stride-0 AP), and then runs the PREVIOUS super-chunk's phase-2
    block. The PE therefore always has phase-2 stream work while the
    exp/convert chain for the next quarter completes - no transition,
    no convert-chain stalls, HAM stays warm.
  * phase 2 = fp8 DoubleRow matmuls (k=256/instruction): stationary =
    w-scaled h chunks [j:256, d:128] (+ an m=1 w column for the
    denominator), moving = adj^T [j:256, r:1024]; 1 output col/cycle
    at 2.4 GHz, LDWEIGHTS hidden behind the matmuls.
  * PSUM budget is exactly 8 banks: 4 numerator accumulators + 2
    denominator accumulators + 2 scratch banks for the h quarters.
  * ws = exp(e + ln(1/8)): the 1/8 keeps w*h inside fp8e4 range; the
    scale cancels in num/den.
  * the device ships the fp16 numerator (transposed [d, r]) and the
    [1, r] denominator row; the host does the final pointwise divide
    and un-transpose (an on-device divide costs 3-7us because DVE
    reciprocal is per-lane serial and den lives on one partition).

Measured numerics: rel err ~9.7e-3 vs fp32 reference (tolerance 2e-2).
"""

import sys

import numpy as np

for _p in ("/opt/trn_rl_repo",):
    try:
        import concourse.bass  # noqa: F401

        break
    except ImportError:
        if _p not in sys.path:
            sys.path.insert(0, _p)

import ml_dtypes

import concourse.bass as bass
import concourse.mybir as mybir
import concourse.tile as tile
from concourse.bass_utils import run_bass_kernel_spmd

dt = mybir.dt
AF = mybir.ActivationFunctionType
PM = mybir.MatmulPerfMode

N = 8192
D = 256
NCORES = 8
RB = N // NCORES  # 1024 output rows per core
W_FREE = 260  # 256 h cols + 1 e col + 3 pad
NJ = N // 128  # 64 j-chunks
NJS = N // 256  # 32 j-super-chunks (DoubleRow k=256)
NJP = NJS // 2  # 16 adj DMA transfers (2 super-chunks each)
LOG_S = float(np.log(1.0 / 8.0))  # global w scale, cancels in num/den

# ---------------------------------------------------------------------------
# walrus in this container accepts at most ONE sync-wait command on several
# instruction structs (Drain, 4-byte self-loading Matmult, ...) while the
# newer Tile scheduler emits more. Split the extras into single-wait
# EventSemaphore prefixes on the same engine (identical semantics).
_ev_counter = [0]


def _legalize_multiwait(nc, max_keep=1):
    for f in nc.m.functions:
        for bb in f.blocks:
            il = bb.instructions
            idx = 0
            while idx < len(il):
                inst = il[idx]
                si = inst.sync_info
                if si is not None and si.on_wait and len(si.on_wait) > max_keep:
                    waits = list(si.on_wait)
                    keep = waits[len(waits) - max_keep :] if max_keep else []
                    extra = waits[: len(waits) - max_keep] if max_keep else waits
                    si.on_wait = keep
                    for w in extra:
                        _ev_counter[0] += 1
                        ev = mybir.InstEventSemaphore(
                            name=f"lgw_{_ev_counter[0]}", ins=[], outs=[]
                        )
                        ev.engine = inst.engine
                        ev.sync_info = mybir.SyncInfo(on_wait=[w], on_update=[])
                        il.insert(idx, ev)
                        idx += 1
                idx += 1


# ---------------------------------------------------------------------------


def _build_program():
    nc = bass.Bass("TRN2", debug=False)

    xT = nc.dram_tensor("xT", [D, N], dt.bfloat16, kind="ExternalInput").ap()
    WTe = nc.dram_tensor("WTe", [D, W_FREE], dt.bfloat16, kind="ExternalInput").ap()
    # adj rows of this core, transposed and packed to fp8 patterns: [j, r]
    adjT8 = nc.dram_tensor("adjT8", [N, RB], dt.float8e4, kind="ExternalInput").ap()
    # transposed numerator: outT[dc, p, r] = num[r, dc*128 + p]; fp16 is
    # plenty (num absmax ~260 << 65504, 5e-4 rel step vs 1e-2 budget)
    outT = nc.dram_tensor("outT", [2, 128, RB], dt.float16, kind="ExternalOutput").ap()
    # softmax denominator row (host performs the final pointwise divide)
    denO = nc.dram_tensor("denO", [1, RB], dt.float32, kind="ExternalOutput").ap()

    XCH = 2048  # x streamed in [128, XCH] bf16 chunks
    NXB = N // XCH  # 4 chunks per i-half
    NCPB = XCH // 128  # 16 j-chunks per x chunk

    with tile.TileContext(nc) as tc:
        with (
            tc.tile_pool(name="xr", bufs=1) as xr_pool,
            tc.tile_pool(name="wte", bufs=1) as wte_pool,
            tc.tile_pool(name="hw8", bufs=1) as hw8_pool,
            tc.tile_pool(name="wcol", bufs=4) as w_pool,
            tc.tile_pool(name="adjr", bufs=16) as adj_pool,
            tc.tile_pool(name="eps", bufs=8) as ep_pool,
        ):
            wte = []
            for ic in range(2):
                t = wte_pool.tile([128, W_FREE], dt.bfloat16, name=f"wte{ic}")
                nc.scalar.dma_start(t, WTe[ic * 128 : (ic + 1) * 128, :])
                wte.append(t)

            # All bulk loads share ONE SWDGE queue so arrival order is
            # exact: first the x chunks feeding the earliest quarters
            # (b=0 split in 128KB pieces for a fast start), then the adj
            # pairs interleaved with the remaining x chunks earliest-
            # deadline-first.
            xr = [[None] * NXB for _ in range(2)]
            at_tiles = [None] * NJP

            def load_x(b):
                for ic in range(2):
                    t = xr_pool.tile(
                        [128, XCH], dt.bfloat16, name=f"xr{ic}_{b}", tag="x", bufs=4
                    )
                    rows = xT[ic * 128 : (ic + 1) * 128]
                    if b == 0:
                        for qq in range(4):
                            nc.gpsimd.dma_start(
                                t[:, qq * 512 : (qq + 1) * 512],
                                rows[:, qq * 512 : (qq + 1) * 512],
                            )
                    else:
                        nc.gpsimd.dma_start(
                            t, rows[:, b * XCH : (b + 1) * XCH]
                        )
                    xr[ic][b] = t

            def load_at(jp):
                at = adj_pool.tile(
                    [128, 2, 2, RB], dt.float8e4, name=f"at{jp}", tag="at"
                )
                src = adjT8[jp * 512 : (jp + 1) * 512, :].rearrange(
                    "(b i p) r -> p b i r", p=128, b=2
                )
                nc.gpsimd.dma_start(at, src)
                at_tiles[jp] = at

            # earliest-deadline-first: the b=0 sub-chunk 0 feeds quarters
            # 0-3, at0 feeds js 0-1 (~11.5us), the rest follow.
            def load_x0_sub(qq):
                for ic in range(2):
                    nc.gpsimd.dma_start(
                        xr[ic][0][:, qq * 512 : (qq + 1) * 512],
                        xT[ic * 128 : (ic + 1) * 128, qq * 512 : (qq + 1) * 512],
                    )

            for ic in range(2):
                xr[ic][0] = xr_pool.tile(
                    [128, XCH], dt.bfloat16, name=f"xr{ic}_0", tag="x", bufs=4
                )
            load_x0_sub(0)
            load_at(0)
            load_x0_sub(1)
            load_at(1)
            load_x0_sub(2)
            load_x0_sub(3)
            load_at(2)
            load_at(3)
            load_x(1)
            load_at(4)
            load_at(5)
            load_x(2)
            load_at(6)
            load_at(7)
            load_x(3)
            for jp in range(8, NJP):
                load_at(jp)

            # fp8 stationaries for phase 2:
            # hw8_all[:, jc, d] = (w*h/8)[j = jc*128 + p, d]
            hw8_all = hw8_pool.tile([128, NJ, 256], dt.float8e4, name="hw8_all")
            # w8_t[:, jc, 0] = (w/8)[j]  (16-byte pair stride for DoubleRow)
            w8_t = hw8_pool.tile([128, NJ, 16], dt.float8e4, name="w8_t")
            w_all = w_pool.tile([128, NJ], dt.float32, name="w_all")
            bias_s = w_pool.tile([128, 1], dt.float32, name="bias_s")
            nc.vector.memset(bias_s, LOG_S)

            # ---- fused loop: PSUM = 2 scratch banks (ph_q) + 4 numerator
            # + 2 denominator accumulator banks = exactly 8.
            with (
                tc.tile_pool(name="ph", bufs=1, space="PSUM") as ph_pool,
                tc.tile_pool(name="acc", bufs=1, space="PSUM") as acc_pool,
            ):
                ph_q = ph_pool.tile([128, 2, 512], dt.float32, name="ph_q")
                # HAM warm-up: dependency-free matmuls on uninitialized SBUF
                # while the first x/adj DMAs are in flight, so the PE clock
                # gate is already at 8/8 when real work starts. Garbage
                # results land in ph_q slot 0 and are overwritten by the
                # first real start=True matmul.
                warm = hw8_pool.tile([128, 640], dt.bfloat16, name="warm")
                nc.gpsimd.memset(warm, 0.0)
                for _ in range(10):
                    nc.tensor.matmul(
                        ph_q[:, 0, :],
                        warm[:, 0:128],
                        warm[:, 128:640],
                        start=True,
                        stop=True,
                    )
                accN = [
                    [
                        acc_pool.tile([128, 512], dt.float32, name=f"accN{dc}_{rh}")
                        for rh in range(2)
                    ]
                    for dc in range(2)
                ]
                accD = [
                    acc_pool.tile([128, 512], dt.float32, name=f"accD{rh}")
                    for rh in range(2)
                ]

                def emit_quarter(q):
                    # h/e matmuls for j-chunks 2q, 2q+1 into the scratch
                    # banks, then ws = exp(e)/8 and the fp8 stationaries.
                    for ic in range(2):
                        for k in range(2):
                            jc = 2 * q + k
                            b, sl = jc // NCPB, bass.ts(jc % NCPB, 128)
                            nc.tensor.matmul(
                                ph_q[:, k, 0:W_FREE],
                                xr[ic][b][:, sl],
                                wte[ic],
                                start=(ic == 0),
                                stop=(ic == 1),
                            )
                    j0 = 2 * q
                    nc.scalar.activation(
                        w_all[:, j0 : j0 + 2],
                        ph_q[:, :, 256],
                        AF.Exp,
                        bias=bias_s[:, 0:1],
                    )
                    nc.scalar.activation(
                        w8_t[:, j0 : j0 + 2, 0], w_all[:, j0 : j0 + 2], AF.Copy
                    )
                    nc.vector.tensor_tensor(
                        hw8_all[:, j0 : j0 + 2, :],
                        ph_q[:, :, 0:256],
                        w_all[:, j0 : j0 + 2].to_broadcast([128, 2, 256]),
                        mybir.AluOpType.mult,
                    )

                def emit_js(js):
                    at = at_tiles[js // 2][:, js % 2]  # [128, 2, RB]
                    st, sp = js == 0, js == NJS - 1
                    order = (
                        [("D", 0), ("D", 1), ("N", 0, 0), ("N", 0, 1),
                         ("N", 1, 0), ("N", 1, 1)]
                        if sp
                        else [("N", 0, 0), ("N", 0, 1), ("N", 1, 0),
                              ("N", 1, 1), ("D", 0), ("D", 1)]
                    )
                    for item in order:
                        if item[0] == "N":
                            dc, rh = item[1], item[2]
                            lhs = hw8_all[
                                :, 2 * js : 2 * js + 2, dc * 128 : (dc + 1) * 128
                            ]
                            nc.tensor.matmul(
                                accN[dc][rh],
                                lhs,
                                at[:, :, rh * 512 : (rh + 1) * 512],
                                start=st,
                                stop=sp,
                                perf_mode=PM.DoubleRow,
                                skip_group_check=True,
                            )
                        else:
                            rh = item[1]
                            nc.tensor.matmul(
                                accD[rh][0:1, :],
                                w8_t[:, 2 * js : 2 * js + 2, 0:1],
                                at[:, :, rh * 512 : (rh + 1) * 512],
                                start=st,
                                stop=sp,
                                perf_mode=PM.DoubleRow,
                                skip_group_check=True,
                            )

                for q in range(NJS):
                    emit_quarter(q)
                    if q >= 1:
                        emit_js(q - 1)
                emit_js(NJS - 1)

                # ---- epilogue: stage PSUM -> SBUF and ship the fp16
                # numerator + the [1, RB] denominator row; the host divides.
                for rh in range(2):
                    dsb = ep_pool.tile([1, 512], dt.float32, name="dsb", tag="dsb")
                    nc.vector.tensor_copy(dsb, accD[rh][0:1, :])
                    nc.sync.dma_start(denO[0:1, rh * 512 : (rh + 1) * 512], dsb)
                    for dc in range(2):
                        ob = ep_pool.tile([128, 512], dt.float16, name="ob", tag="ob")
                        if (rh + dc) % 2 == 0:
                            nc.vector.tensor_copy(ob, accN[dc][rh])
                        else:
                            nc.scalar.activation(ob, accN[dc][rh], AF.Copy)
                        eng = nc.sync if dc == 0 else nc.scalar
                        eng.dma_start(outT[dc, :, rh * 512 : (rh + 1) * 512], ob)

    _legalize_multiwait(nc, max_keep=1)
    return nc


_CACHED = {}


def _prep_inputs(x, adj, W, a):
    xT = np.ascontiguousarray(x.T).astype(ml_dtypes.bfloat16)
    WTe = np.zeros((D, W_FREE), dtype=np.float32)
    WTe[:, :256] = W.T
    WTe[:, 256] = (W.T.astype(np.float64) @ a[256:].astype(np.float64)).astype(
        np.float32
    )
    WTe = WTe.astype(ml_dtypes.bfloat16)
    # adjacency -> fp8e4 bit patterns (0x00 / 0x38 == 1.0), transposed per core
    adj8 = np.where(adj != 0, np.uint8(0x38), np.uint8(0)).view(ml_dtypes.float8_e4m3)
    in_maps = []
    for c in range(NCORES):
        adjT8_c = np.ascontiguousarray(adj8[c * RB : (c + 1) * RB, :].T)
        in_maps.append({"xT": xT, "WTe": WTe, "adjT8": adjT8_c})
    return in_maps


def _run(in_maps, **kw):
    if "nc" not in _CACHED:
        _CACHED["nc"] = _build_program()
    # The device occasionally comes up wedged (NRT_EXEC_UNIT_UNRECOVERABLE)
    # from a previous process; one retry after a short pause recovers it.
    import time as _time

    last_err = None
    for attempt in range(3):
        try:
            return run_bass_kernel_spmd(
                _CACHED["nc"], in_maps, core_ids=list(range(NCORES)), **kw
            )
        except Exception as e:  # noqa: BLE001
            last_err = e
            if "UNRECOVERABLE" not in str(e) and "UNAVAILABLE" not in str(e):
                raise
            _time.sleep(3.0)
    raise last_err


def _assemble(results):
    blocks = []
    for r in results:
        t = np.asarray(r["outT"], dtype=np.float32)  # [2, 128, RB] numerator
        den = r["denO"].reshape(RB)  # [RB] softmax denominator
        num = t.transpose(2, 0, 1).reshape(RB, D)
        blocks.append(num / den[:, None])
    return np.concatenate(blocks, axis=0).astype(np.float32, copy=False)


def kernel(x, adj, W, a):
    in_maps = _prep_inputs(x, adj, W, a)
    res = _run(in_maps)
    return _assemble(res.results)


# revision 27
# speedup vs baseline: 1.3115x; 1.3115x over previous
"""GAT layer kernel for Trainium2, sharded across 8 NeuronCores.

Math: reference computes
    h = x @ W.T;  e_ij = (h @ a1)[i] + (h @ a2)[j];  mask by adj;
    softmax over j; out = attn @ h.
Because e_i is constant along the softmax axis it cancels, so with
w_j = exp(h_j . a2):
    out[i] = sum_j adj[i,j] * w_j * h[j] / sum_j adj[i,j] * w_j
a1 is mathematically irrelevant.

Design (v10, fused single pass):
  * adjacency is packed host-side to fp8e4 BIT PATTERNS (0x00 / 0x38 =
    1.0): 1 byte/entry (8.4 MB/core vs 33.5 int32), streamed pure-copy
    on SWDGE, interleaved just-in-time with the bf16 x stream.
  * ONE fused loop over 32 j-super-chunks: each iteration computes the
    next quarter of h (2 j-chunks, bf16 matmuls into a 2-bank PSUM
    scratch), exps the e column read straight out of PSUM, converts
    w*h to the fp8 stationaries in a single DVE op (w broadcast via a
    stride-0 AP), and then runs the PREVIOUS super-chunk's phase-2
    block. The PE therefore always has phase-2 stream work while the
    exp/convert chain for the next quarter completes - no transition,
    no convert-chain stalls, HAM stays warm.
  * phase 2 = fp8 DoubleRow matmuls (k=256/instruction): stationary =
    w-scaled h chunks [j:256, d:128] (+ an m=1 w column for the
    denominator), moving = adj^T [j:256, r:1024]; 1 output col/cycle
    at 2.4 GHz, LDWEIGHTS hidden behind the matmuls.
  * PSUM budget is exactly 8 banks: 4 numerator accumulators + 2
    denominator accumulators + 2 scratch banks for the h quarters.
  * ws = exp(e + ln(1/8)): the 1/8 keeps w*h inside fp8e4 range; the
    scale cancels in num/den.
  * the device ships the fp16 numerator (transposed [d, r]) and the
    [1, r] denominator row; the host does the final pointwise divide
    and un-transpose (an on-device divide costs 3-7us because DVE
    reciprocal is per-lane serial and den lives on one partition).

Measured numerics: rel err ~9.7e-3 vs fp32 reference (tolerance 2e-2).
"""

import sys

import numpy as np

for _p in ("/opt/trn_rl_repo",):
    try:
        import concourse.bass  # noqa: F401

        break
    except ImportError:
        if _p not in sys.path:
            sys.path.insert(0, _p)

import ml_dtypes

import concourse.bass as bass
import concourse.mybir as mybir
import concourse.tile as tile
from concourse.bass_utils import run_bass_kernel_spmd

dt = mybir.dt
AF = mybir.ActivationFunctionType
PM = mybir.MatmulPerfMode

N = 8192
D = 256
NCORES = 8
RB = N // NCORES  # 1024 output rows per core
W_FREE = 260  # 256 h cols + 1 e col + 3 pad
NJ = N // 128  # 64 j-chunks
NJS = N // 256  # 32 j-super-chunks (DoubleRow k=256)
NJP = NJS // 2  # 16 adj DMA transfers (2 super-chunks each)
LOG_S = float(np.log(1.0 / 8.0))  # global w scale, cancels in num/den

# ---------------------------------------------------------------------------
# walrus in this container accepts at most ONE sync-wait command on several
# instruction structs (Drain, 4-byte self-loading Matmult, ...) while the
# newer Tile scheduler emits more. Split the extras into single-wait
# EventSemaphore prefixes on the same engine (identical semantics).
_ev_counter = [0]


def _legalize_multiwait(nc, max_keep=1):
    for f in nc.m.functions:
        for bb in f.blocks:
            il = bb.instructions
            idx = 0
            while idx < len(il):
                inst = il[idx]
                si = inst.sync_info
                if si is not None and si.on_wait and len(si.on_wait) > max_keep:
                    waits = list(si.on_wait)
                    keep = waits[len(waits) - max_keep :] if max_keep else []
                    extra = waits[: len(waits) - max_keep] if max_keep else waits
                    si.on_wait = keep
                    for w in extra:
                        _ev_counter[0] += 1
                        ev = mybir.InstEventSemaphore(
                            name=f"lgw_{_ev_counter[0]}", ins=[], outs=[]
                        )
                        ev.engine = inst.engine
                        ev.sync_info = mybir.SyncInfo(on_wait=[w], on_update=[])
                        il.insert(idx, ev)
                        idx += 1
                idx += 1


# ---------------------------------------------------------------------------


def _build_program():
    nc = bass.Bass("TRN2", debug=False)

    xT = nc.dram_tensor("xT", [D, N], dt.bfloat16, kind="ExternalInput").ap()
    WTe = nc.dram_tensor("WTe", [D, W_FREE], dt.bfloat16, kind="ExternalInput").ap()
    # adj rows of this core, transposed and packed to fp8 patterns: [j, r]
    adjT8 = nc.dram_tensor("adjT8", [N, RB], dt.float8e4, kind="ExternalInput").ap()
    # transposed numerator: outT[dc, p, r] = num[r, dc*128 + p]; fp16 is
    # plenty (num absmax ~260 << 65504, 5e-4 rel step vs 1e-2 budget)
    outT = nc.dram_tensor("outT", [2, 128, RB], dt.float16, kind="ExternalOutput").ap()
    # softmax denominator row (host performs the final pointwise divide)
    denO = nc.dram_tensor("denO", [1, RB], dt.float32, kind="ExternalOutput").ap()

    XCH = 2048  # x streamed in [128, XCH] bf16 chunks
    NXB = N // XCH  # 4 chunks per i-half
    NCPB = XCH // 128  # 16 j-chunks per x chunk

    with tile.TileContext(nc) as tc:
        with (
            tc.tile_pool(name="xr", bufs=1) as xr_pool,
            tc.tile_pool(name="wte", bufs=1) as wte_pool,
            tc.tile_pool(name="hw8", bufs=1) as hw8_pool,
            tc.tile_pool(name="wcol", bufs=4) as w_pool,
            tc.tile_pool(name="adjr", bufs=16) as adj_pool,
            tc.tile_pool(name="eps", bufs=8) as ep_pool,
        ):
            wte = []
            for ic in range(2):
                t = wte_pool.tile([128, W_FREE], dt.bfloat16, name=f"wte{ic}")
                nc.scalar.dma_start(t, WTe[ic * 128 : (ic + 1) * 128, :])
                wte.append(t)

            # All bulk loads share ONE SWDGE queue so arrival order is
            # exact: first the x chunks feeding the earliest quarters
            # (b=0 split in 128KB pieces for a fast start), then the adj
            # pairs interleaved with the remaining x chunks earliest-
            # deadline-first.
            xr = [[None] * NXB for _ in range(2)]
            at_tiles = [None] * NJP

            def load_x(b):
                for ic in range(2):
                    t = xr_pool.tile(
                        [128, XCH], dt.bfloat16, name=f"xr{ic}_{b}", tag="x", bufs=4
                    )
                    rows = xT[ic * 128 : (ic + 1) * 128]
                    if b == 0:
                        for qq in range(4):
                            nc.gpsimd.dma_start(
                                t[:, qq * 512 : (qq + 1) * 512],
                                rows[:, qq * 512 : (qq + 1) * 512],
                            )
                    else:
                        nc.gpsimd.dma_start(
                            t, rows[:, b * XCH : (b + 1) * XCH]
                        )
                    xr[ic][b] = t

            def load_at(jp):
                at = adj_pool.tile(
                    [128, 2, 2, RB], dt.float8e4, name=f"at{jp}", tag="at"
                )
                src = adjT8[jp * 512 : (jp + 1) * 512, :].rearrange(
                    "(b i p) r -> p b i r", p=128, b=2
                )
                nc.gpsimd.dma_start(at, src)
                at_tiles[jp] = at

            # earliest-deadline-first: the b=0 sub-chunk 0 feeds quarters
            # 0-3, at0 feeds js 0-1 (~11.5us), the rest follow.
            def load_x0_sub(qq):
                for ic in range(2):
                    nc.gpsimd.dma_start(
                        xr[ic][0][:, qq * 512 : (qq + 1) * 512],
                        xT[ic * 128 : (ic + 1) * 128, qq * 512 : (qq + 1) * 512],
                    )

            for ic in range(2):
                xr[ic][0] = xr_pool.tile(
                    [128, XCH], dt.bfloat16, name=f"xr{ic}_0", tag="x", bufs=4
                )
            load_x0_sub(0)
            load_at(0)
            load_x0_sub(1)
            load_at(1)
            load_x0_sub(2)
            load_x0_sub(3)
            load_at(2)
            load_at(3)
            load_x(1)
            load_at(4)
            load_at(5)
            load_x(2)
            load_at(6)
            load_at(7)
            load_x(3)
            for jp in range(8, NJP):
                load_at(jp)

            # fp8 stationaries for phase 2:
            # hw8_all[:, jc, d] = (w*h/8)[j = jc*128 + p, d]
            hw8_all = hw8_pool.tile([128, NJ, 256], dt.float8e4, name="hw8_all")
            # w8_t[:, jc, 0] = (w/8)[j]  (16-byte pair stride for DoubleRow)
            w8_t = hw8_pool.tile([128, NJ, 16], dt.float8e4, name="w8_t")
            w_all = w_pool.tile([128, NJ], dt.float32, name="w_all")
            bias_s = w_pool.tile([128, 1], dt.float32, name="bias_s")
            nc.vector.memset(bias_s, LOG_S)

            # ---- fused loop: PSUM = 2 scratch banks (ph_q) + 4 numerator
            # + 2 denominator accumulator banks = exactly 8.
            with (
                tc.tile_pool(name="ph", bufs=1, space="PSUM") as ph_pool,
                tc.tile_pool(name="acc", bufs=1, space="PSUM") as acc_pool,
            ):
                ph_q = ph_pool.tile([128, 2, 512], dt.float32, name="ph_q")
                # HAM warm-up: dependency-free matmuls on uninitialized SBUF
                # while the first x/adj DMAs are in flight, so the PE clock
                # gate is already at 8/8 when real work starts. Garbage
                # results land in ph_q slot 0 and are overwritten by the
                # first real start=True matmul.
                warm = hw8_pool.tile([128, 640], dt.bfloat16, name="warm")
                # memset on DVE: the gpsimd queue is busy emitting all the
                # SWDGE DMA descriptors for ~30us - anything queued behind
                # them would delay the warm-up matmuls (and the whole PE
                # timeline) by that much.
                nc.vector.memset(warm, 0.0)
                for _ in range(10):
                    nc.tensor.matmul(
                        ph_q[:, 0, :],
                        warm[:, 0:128],
                        warm[:, 128:640],
                        start=True,
                        stop=True,
                    )
                accN = [
                    [
                        acc_pool.tile([128, 512], dt.float32, name=f"accN{dc}_{rh}")
                        for rh in range(2)
                    ]
                    for dc in range(2)
                ]
                accD = [
                    acc_pool.tile([128, 512], dt.float32, name=f"accD{rh}")
                    for rh in range(2)
                ]

                def emit_quarter(q):
                    # h/e matmuls for j-chunks 2q, 2q+1 into the scratch
                    # banks, then ws = exp(e)/8 and the fp8 stationaries.
                    for ic in range(2):
                        for k in range(2):
                            jc = 2 * q + k
                            b, sl = jc // NCPB, bass.ts(jc % NCPB, 128)
                            nc.tensor.matmul(
                                ph_q[:, k, 0:W_FREE],
                                xr[ic][b][:, sl],
                                wte[ic],
                                start=(ic == 0),
                                stop=(ic == 1),
                            )
                    j0 = 2 * q
                    nc.scalar.activation(
                        w_all[:, j0 : j0 + 2],
                        ph_q[:, :, 256],
                        AF.Exp,
                        bias=bias_s[:, 0:1],
                    )
                    nc.scalar.activation(
                        w8_t[:, j0 : j0 + 2, 0], w_all[:, j0 : j0 + 2], AF.Copy
                    )
                    nc.vector.tensor_tensor(
                        hw8_all[:, j0 : j0 + 2, :],
                        ph_q[:, :, 0:256],
                        w_all[:, j0 : j0 + 2].to_broadcast([128, 2, 256]),
                        mybir.AluOpType.mult,
                    )

                def emit_js(js):
                    at = at_tiles[js // 2][:, js % 2]  # [128, 2, RB]
                    st, sp = js == 0, js == NJS - 1
                    order = (
                        [("D", 0), ("D", 1), ("N", 0, 0), ("N", 0, 1),
                         ("N", 1, 0), ("N", 1, 1)]
                        if sp
                        else [("N", 0, 0), ("N", 0, 1), ("N", 1, 0),
                              ("N", 1, 1), ("D", 0), ("D", 1)]
                    )
                    for item in order:
                        if item[0] == "N":
                            dc, rh = item[1], item[2]
                            lhs = hw8_all[
                                :, 2 * js : 2 * js + 2, dc * 128 : (dc + 1) * 128
                            ]
                            nc.tensor.matmul(
                                accN[dc][rh],
                                lhs,
                                at[:, :, rh * 512 : (rh + 1) * 512],
                                start=st,
                                stop=sp,
                                perf_mode=PM.DoubleRow,
                                skip_group_check=True,
                            )
                        else:
                            rh = item[1]
                            nc.tensor.matmul(
                                accD[rh][0:1, :],
                                w8_t[:, 2 * js : 2 * js + 2, 0:1],
                                at[:, :, rh * 512 : (rh + 1) * 512],
                                start=st,
                                stop=sp,
                                perf_mode=PM.DoubleRow,
                                skip_group_check=True,
                            )

                for q in range(NJS):
                    emit_quarter(q)
                    if q >= 1:
                        emit_js(q - 1)
                emit_js(NJS - 1)

                # ---- epilogue: stage PSUM -> SBUF and ship the fp16
                # numerator + the [1, RB] denominator row; the host divides.
                for rh in range(2):
                    dsb = ep_pool.tile([1, 512], dt.float32, name="dsb", tag="dsb")
                    nc.vector.tensor_copy(dsb, accD[rh][0:1, :])
                    nc.sync.dma_start(denO[0:1, rh * 512 : (rh + 1) * 512], dsb)
                    for dc in range(2):
                        ob = ep_pool.tile([128, 512], dt.float16, name="ob", tag="ob")
                        if (rh + dc) % 2 == 0:
                            nc.vector.tensor_copy(ob, accN[dc][rh])
                        else:
                            nc.scalar.activation(ob, accN[dc][rh], AF.Copy)
                        eng = nc.sync if dc == 0 else nc.scalar
                        eng.dma_start(outT[dc, :, rh * 512 : (rh + 1) * 512], ob)

    _legalize_multiwait(nc, max_keep=1)
    return nc


_CACHED = {}


def _prep_inputs(x, adj, W, a):
    xT = np.ascontiguousarray(x.T).astype(ml_dtypes.bfloat16)
    WTe = np.zeros((D, W_FREE), dtype=np.float32)
    WTe[:, :256] = W.T
    WTe[:, 256] = (W.T.astype(np.float64) @ a[256:].astype(np.float64)).astype(
        np.float32
    )
    WTe = WTe.astype(ml_dtypes.bfloat16)
    # adjacency -> fp8e4 bit patterns (0x00 / 0x38 == 1.0), transposed per core
    adj8 = np.where(adj != 0, np.uint8(0x38), np.uint8(0)).view(ml_dtypes.float8_e4m3)
    in_maps = []
    for c in range(NCORES):
        adjT8_c = np.ascontiguousarray(adj8[c * RB : (c + 1) * RB, :].T)
        in_maps.append({"xT": xT, "WTe": WTe, "adjT8": adjT8_c})
    return in_maps


def _run(in_maps, **kw):
    if "nc" not in _CACHED:
        _CACHED["nc"] = _build_program()
    # The device occasionally comes up wedged (NRT_EXEC_UNIT_UNRECOVERABLE)
    # from a previous process; one retry after a short pause recovers it.
    import time as _time

    last_err = None
    for attempt in range(3):
        try:
            return run_bass_kernel_spmd(
                _CACHED["nc"], in_maps, core_ids=list(range(NCORES)), **kw
            )
        except Exception as e:  # noqa: BLE001
            last_err = e
            if "UNRECOVERABLE" not in str(e) and "UNAVAILABLE" not in str(e):
                raise
            _time.sleep(3.0)
    raise last_err


def _assemble(results):
    blocks = []
    for r in results:
        t = np.asarray(r["outT"], dtype=np.float32)  # [2, 128, RB] numerator
        den = r["denO"].reshape(RB)  # [RB] softmax denominator
        num = t.transpose(2, 0, 1).reshape(RB, D)
        blocks.append(num / den[:, None])
    return np.concatenate(blocks, axis=0).astype(np.float32, copy=False)


def kernel(x, adj, W, a):
    in_maps = _prep_inputs(x, adj, W, a)
    res = _run(in_maps)
    return _assemble(res.results)
